# revision 15
# baseline (speedup 1.0000x reference)
"""ACNN sparse-attention Trainium2 kernel (8 NeuronCores, pure data parallel).

Reference computation (per batch b, token s):
  windows[s, w]   = x[s + w - 2]           (zero-padded outside the sequence)
  q               = x[s] @ Wq
  kp[s']          = x[s'] @ Wk             (zero rows stay zero)
  scores[s, w]    = sum_h v[h] * tanh(kp[s + w - 2, h] + q[s, h])
  att             = softmax_w(scores)
  out[s]          = sum_w att[s, w] * (x[s + w - 2] @ cnn_W[w*F:(w+1)*F]) + b

Sharding: batch 16 -> 2 per core. Everything on-device is computed from
x^T ([F, tokens], bf16) so all matmul contractions sit on the partition dim;
the W=5 window shifts become free-dim column shifts of x^T / kp^T.
"""
import sys

sys.path.insert(0, "/opt/trn_rl_repo")

from contextlib import ExitStack

import numpy as np
import ml_dtypes

import concourse.bass as bass
import concourse.tile as tile
from concourse import mybir
from concourse.bass_utils import run_bass_kernel_spmd

def _install_ntff_hook_shim():
    """The image's antenv package lacks axon_hooks; recreate it so
    run_bass_kernel_spmd(trace=True) can capture NTFF profiles."""
    import types

    if "antenv.axon_hooks" in sys.modules:
        return
    mod = types.ModuleType("antenv.axon_hooks")
    mod._hook = None
    mod.set_axon_ntff_profile_hook = lambda h: setattr(mod, "_hook", h)
    mod.get_axon_ntff_profile_hook = lambda: mod._hook
    sys.modules["antenv.axon_hooks"] = mod
    try:
        sys.path.insert(0, "/root/.axon_site/trn_agent_boot")
        import trn_boot

        hook = trn_boot._ntff_profile_via_ctypes("/opt/axon/libaxon_pjrt.so")
        if hook is not None:
            mod._hook = hook
    except Exception:
        pass


_install_ntff_hook_shim()

F32 = mybir.dt.float32
BF16 = mybir.dt.bfloat16
AF = mybir.ActivationFunctionType
AX = mybir.AxisListType

B, S, F, H, W = 16, 2048, 512, 512, 5
PAD = W // 2
NCORES = 8
BPC = B // NCORES          # batches per core
T = BPC * S                # tokens per core (4096)
TB = 512                   # token block
NB = T // TB               # 8 blocks per core
NBSEG = S // TB            # 4 blocks per segment (batch)
FC = F // 128              # 4 f-chunks
HC = H // 128              # 4 h-chunks
SEGW = S + 2 * PAD         # padded segment width in SBUF (2052)
XW = BPC * SEGW            # padded token width in SBUF (4104)

_MAX_CTRL_WAITS = 1


def _patch_tile_drain():
    """walrus rejects >4 sem waits on one CTRL; spread the TileContext exit
    drain's waits over extra drain instructions."""
    if getattr(tile.TileContext, "_acnn_drain_patched", False):
        return
    from concourse.vector_clock import ScopedClock

    def _drain_and_barrier(self, tick_clock, wait_clock):
        nc = self.nc
        drain_inst = nc.sync.drain()
        wait_clock.add_sem_waits(
            drain_inst.ins, ScopedClock({None: tick_clock.global_clock})
        )
        si = drain_inst.ins.sync_info
        waits = list(si.on_wait) if si and si.on_wait else []
        if len(waits) > _MAX_CTRL_WAITS:
            si.on_wait = waits[:_MAX_CTRL_WAITS]
            rest = waits[_MAX_CTRL_WAITS:]
            for i in range(0, len(rest), _MAX_CTRL_WAITS):
                extra = nc.sync.drain()
                esi = extra.ins.sync_info
                chunk = rest[i : i + _MAX_CTRL_WAITS]
                if esi is None:
                    extra.ins.sync_info = type(si)(on_wait=chunk, on_update=[])
                else:
                    esi.on_wait = chunk
        nc.all_engine_barrier()
        popped = nc._tile_sem_poison_stack.pop()
        assert popped is self._sem_poison
        nc.clear_and_free_semaphores(list(self.sems.allocated().values()))
        nc.all_engine_barrier()

    tile.TileContext._drain_and_barrier = _drain_and_barrier
    tile.TileContext._acnn_drain_patched = True


def _split_multi_waits(nc, max_waits=_MAX_CTRL_WAITS):
    """This walrus build rejects >1 sem wait per compute instruction; hoist
    overflow waits onto same-engine NoOps placed just before (engine queues
    are strict FIFO, so the gating is equivalent)."""
    n_split = 0
    for f in nc.m.functions:
        for bb in f.blocks:
            new = []
            for inst in bb.instructions:
                si = inst.sync_info
                waits = list(si.on_wait) if si is not None and si.on_wait else []
                if len(waits) > max_waits:
                    extra = waits[: len(waits) - max_waits]
                    si.on_wait = waits[len(waits) - max_waits:]
                    for i in range(0, len(extra), max_waits):
                        nop = mybir.InstNoOp(
                            name=f"{inst.name}-xw{i}",
                            engine=inst.engine,
                            sync_info=mybir.SyncInfo(
                                on_wait=extra[i : i + max_waits], on_update=[]
                            ),
                            bass_nofuse=True,
                            ins=[],
                            outs=[],
                        )
                        new.append(nop)
                    n_split += 1
                new.append(inst)
            bb.instructions[:] = new
    return n_split


def build():
    _patch_tile_drain()
    nc = bass.Bass(trn_type="TRN2")

    AXW = 520  # per-f-chunk region width in the ax tiles (512 + shift fringe)

    xT = nc.dram_tensor("xT", [F, T], BF16, kind="ExternalInput")
    wq = nc.dram_tensor("wq", [F, H], BF16, kind="ExternalInput")
    wk = nc.dram_tensor("wk", [F, H], BF16, kind="ExternalInput")
    cw = nc.dram_tensor("cw", [W * F, H], BF16, kind="ExternalInput")
    vT = nc.dram_tensor("vT", [128, HC], BF16, kind="ExternalInput")
    bb = nc.dram_tensor("bb", [128, H], F32, kind="ExternalInput")
    ident = nc.dram_tensor("ident", [128, 128], BF16, kind="ExternalInput")
    out = nc.dram_tensor("out", [T, H], F32, kind="ExternalOutput")

    with ExitStack() as ctx:
        tc = ctx.enter_context(tile.TileContext(nc))

        const = ctx.enter_context(tc.tile_pool(name="const", bufs=1))
        wq_sb = const.tile([128, FC * H], BF16, tag="wq")
        wk_sb = const.tile([128, FC * H], BF16, tag="wk")
        cw_sb = const.tile([128, W * FC * H], BF16, tag="cw")
        vT_sb = const.tile([128, HC], BF16, tag="vT")
        bb_sb = const.tile([128, H], F32, tag="bb")
        id_sb = const.tile([128, 128], BF16, tag="ident")
        xt_sb = const.tile([128, FC * XW], BF16, tag="xt")
        kp_sb = [const.tile([128, XW], BF16, name=f"kp{hc}", tag=f"kp{hc}") for hc in range(HC)]
        q_sb = [const.tile([128, T], BF16, name=f"q{hc}", tag=f"q{hc}") for hc in range(HC)]

        def dma_xt(fc, seg, half=None):
            o0, o1 = (0, S) if half is None else (half * (S // 2), (half + 1) * (S // 2))
            nc.sync.dma_start(
                xt_sb[:, fc * XW + seg * SEGW + PAD + o0: fc * XW + seg * SEGW + PAD + o1],
                xT[fc * 128:(fc + 1) * 128, seg * S + o0: seg * S + o1],
            )

        # issue order matters: the first qkp matmuls need xt(seg0) + wq
        for fc in range(FC):
            dma_xt(fc, 0, 0)
            nc.sync.dma_start(wq_sb[:, fc * H:(fc + 1) * H], wq[fc * 128:(fc + 1) * 128, :])
        for fc in range(FC):
            dma_xt(fc, 0, 1)
        for fc in range(FC):
            nc.sync.dma_start(wk_sb[:, fc * H:(fc + 1) * H], wk[fc * 128:(fc + 1) * 128, :])
            dma_xt(fc, 1)
        nc.sync.dma_start(vT_sb[:], vT[:])
        nc.sync.dma_start(bb_sb[:], bb[:])
        nc.sync.dma_start(id_sb[:], ident[:])
        for w in range(W):
            for fc in range(FC):
                r0 = w * F + fc * 128
                c0 = (w * FC + fc) * H
                nc.sync.dma_start(cw_sb[:, c0:c0 + H], cw[r0:r0 + 128, :])
        # zero the halo columns of x^T and kp^T
        for seg in range(BPC):
            for fc in range(FC):
                b0 = fc * XW + seg * SEGW
                nc.vector.memset(xt_sb[:, b0:b0 + PAD], 0.0)
                nc.vector.memset(xt_sb[:, b0 + PAD + S:b0 + SEGW], 0.0)
            for hc in range(HC):
                nc.vector.memset(kp_sb[hc][:, seg * SEGW: seg * SEGW + PAD], 0.0)
                nc.vector.memset(kp_sb[hc][:, seg * SEGW + PAD + S: (seg + 1) * SEGW], 0.0)

        qkp_ps = ctx.enter_context(tc.tile_pool(name="qkp_ps", bufs=2, space="PSUM"))
        sc_ps_pool = ctx.enter_context(tc.tile_pool(name="sc_ps", bufs=1, space="PSUM"))
        tr_ps_pool = ctx.enter_context(tc.tile_pool(name="tr_ps", bufs=1, space="PSUM"))
        out_ps_pool = ctx.enter_context(tc.tile_pool(name="out_ps", bufs=1, space="PSUM"))

        argp = ctx.enter_context(tc.tile_pool(name="argp", bufs=2))
        thp = ctx.enter_context(tc.tile_pool(name="thp", bufs=5))
        smp = ctx.enter_context(tc.tile_pool(name="smp", bufs=4))
        attp = ctx.enter_context(tc.tile_pool(name="attp", bufs=2))
        bcp = ctx.enter_context(tc.tile_pool(name="bcp", bufs=4))
        axp = ctx.enter_context(tc.tile_pool(name="axp", bufs=3))
        outp = ctx.enter_context(tc.tile_pool(name="outp", bufs=4))

        def xcol0(b):
            return (b // NBSEG) * SEGW + PAD + (b % NBSEG) * TB

        th_tiles = {}
        attT_tiles = {}
        sc_tiles = {}

        def emit_qkp(b):
            xc = xcol0(b)
            qc = b * TB
            for hc in range(HC):
                for which, w_sb, dst, dc in (
                    (0, wq_sb, q_sb[hc], qc),
                    (1, wk_sb, kp_sb[hc], xc),
                ):
                    ps = qkp_ps.tile([128, TB], F32, name="ps", tag="qkp")
                    for fc in range(FC):
                        nc.tensor.matmul(
                            ps[:],
                            w_sb[:, fc * H + hc * 128: fc * H + (hc + 1) * 128],
                            xt_sb[:, fc * XW + xc: fc * XW + xc + TB],
                            start=(fc == 0),
                            stop=(fc == FC - 1),
                        )
                    nc.scalar.activation(dst[:, dc:dc + TB], ps[:], AF.Copy)

        def emit_addstanh(b):
            xc = xcol0(b)
            qc = b * TB
            ths = []
            for hc in range(HC):
                arg = argp.tile([128, W * TB], BF16, name="arg", tag="arg")
                for w in range(W):
                    nc.vector.tensor_add(
                        arg[:, w * TB:(w + 1) * TB],
                        kp_sb[hc][:, xc - PAD + w: xc - PAD + w + TB],
                        q_sb[hc][:, qc:qc + TB],
                    )
                th = thp.tile([128, W * TB], BF16, name="th", tag="th")
                nc.scalar.activation(th[:], arg[:], AF.Tanh)
                ths.append(th)
            th_tiles[b] = ths

        def emit_scores(b):
            sc_ps = sc_ps_pool.tile([128, 32], F32, name="sc", tag="sc")
            ths = th_tiles.pop(b)
            for hc in range(HC):
                th = ths[hc]
                for w in range(W):
                    for g in range(4):
                        col = g * 8 + w
                        nc.tensor.matmul(
                            sc_ps[:, col:col + 1],
                            th[:, w * TB + g * 128: w * TB + (g + 1) * 128],
                            vT_sb[:, hc:hc + 1],
                            start=(hc == 0 and w == 0 and g == 0),
                            stop=(hc == HC - 1 and w == W - 1 and g == 3),
                            skip_group_check=True,
                        )
            # softmax over W=5 (scores bounded by |v|_1 ~ 8, no max-sub needed)
            attT = attp.tile([W, TB], BF16, name="attT", tag="attT")
            for g in range(4):
                ex = smp.tile([128, W], F32, name="ex", tag="ex")
                nc.scalar.activation(ex[:], sc_ps[:, g * 8: g * 8 + W], AF.Exp)
                sm = smp.tile([128, 1], F32, name="sm", tag="sm")
                nc.vector.reduce_sum(sm[:], ex[:], AX.X)
                rc = smp.tile([128, 1], F32, name="rc", tag="rc")
                nc.vector.reciprocal(rc[:], sm[:])
                attg = smp.tile([128, W], BF16, name="attg", tag="attg")
                nc.vector.tensor_scalar_mul(attg[:], ex[:], rc[:])
                tp = tr_ps_pool.tile([128, 128], BF16, name="tp", tag="tr")
                nc.tensor.transpose(tp[0:W, :], attg[:], id_sb[:])
                nc.scalar.activation(attT[:, g * 128:(g + 1) * 128], tp[0:W, :], AF.Copy)
            attT_tiles[b] = attT

        def emit_cnn(b):
            xc = xcol0(b)
            qc = b * TB
            attT = attT_tiles.pop(b)
            ops = [out_ps_pool.tile([128, H], F32, name=f"op{g}", tag=f"op{g}") for g in range(4)]
            abs_ = []
            for w in range(W):
                # att row w broadcast, written at column offset w so the
                # merged multiply below reads x^T at an even (2x-mode) offset
                ab = bcp.tile([128, AXW], BF16, name="ab", tag="ab")
                nc.sync.dma_start(
                    ab[:, w:w + TB],
                    attT[w:w + 1, :].rearrange("p (r c) -> p r c", r=1)
                    .broadcast_to((1, 128, TB)),
                )
                abs_.append(ab)
            for w in range(W):
                ab = abs_[w]
                ax = axp.tile([128, FC * AXW], BF16, name="ax", tag="ax")
                # ax[:, fc, j] = xt[:, fc, xc-2+j] * att_w[j-w]   (j in [0,516))
                nc.vector.tensor_tensor(
                    ax.rearrange("p (f c) -> p f c", f=FC)[:, :, 0:516],
                    xt_sb.rearrange("p (f c) -> p f c", f=FC)[:, :, xc - PAD: xc - PAD + 516],
                    ab[:, 0:516].rearrange("p (r c) -> p r c", r=1)
                    .broadcast_to((128, FC, 516)),
                    mybir.AluOpType.mult,
                )
                for fc in range(FC):
                    for g in range(4):
                        nc.tensor.matmul(
                            ops[g][:],
                            ax[:, fc * AXW + w + g * 128: fc * AXW + w + (g + 1) * 128],
                            cw_sb[:, (w * FC + fc) * H:(w * FC + fc + 1) * H],
                            start=(w == 0 and fc == 0),
                            stop=(w == W - 1 and fc == FC - 1),
                        )
            for g in range(4):
                ot = outp.tile([128, H], F32, name="ot", tag="ot")
                nc.vector.tensor_add(ot[:], ops[g][:], bb_sb[:])
                nc.sync.dma_start(out[qc + g * 128: qc + (g + 1) * 128, :], ot[:])

        # software-pipelined emission: per-engine program order is chosen so
        # ready PE work (next qkp / cnn of block b) is never queued behind
        # PE work that waits on the softmax chain of a newer block.
        emit_qkp(0)
        emit_qkp(1)
        emit_qkp(2)
        emit_addstanh(0)
        emit_scores(0)
        for b in range(NB):
            if b + 3 < NB:
                emit_qkp(b + 3)
            if b + 1 < NB:
                emit_addstanh(b + 1)
                emit_scores(b + 1)
            emit_cnn(b)

    _split_multi_waits(nc)
    return nc


_NC_CACHE = None


def _get_nc():
    global _NC_CACHE
    if _NC_CACHE is None:
        _NC_CACHE = build()
    return _NC_CACHE


def _prep_in_maps(embeds_output, Wq, Wk, v_att, cnn_W, cnn_b):
    bf = ml_dtypes.bfloat16
    wq = np.ascontiguousarray(Wq, dtype=np.float32).astype(bf)
    wk = np.ascontiguousarray(Wk, dtype=np.float32).astype(bf)
    cw = np.ascontiguousarray(cnn_W, dtype=np.float32).astype(bf)
    vT = np.ascontiguousarray(
        np.asarray(v_att, dtype=np.float32).reshape(HC, 128).T
    ).astype(bf)
    bb = np.ascontiguousarray(
        np.broadcast_to(np.asarray(cnn_b, dtype=np.float32)[None, :], (128, H))
    )
    ident = np.eye(128, dtype=np.float32).astype(bf)

    x = np.asarray(embeds_output, dtype=np.float32)
    in_maps = []
    for c in range(NCORES):
        shard = x[c * BPC:(c + 1) * BPC]                  # [BPC, S, F]
        xT = shard.transpose(2, 0, 1).reshape(F, T)       # [F, BPC*S]
        in_maps.append(
            {
                "xT": np.ascontiguousarray(xT).astype(bf),
                "wq": wq,
                "wk": wk,
                "cw": cw,
                "vT": vT,
                "bb": bb,
                "ident": ident,
            }
        )
    return in_maps


def kernel(embeds_output, Wq, Wk, v_att, cnn_W, cnn_b, **run_kwargs):
    nc = _get_nc()
    in_maps = _prep_in_maps(embeds_output, Wq, Wk, v_att, cnn_W, cnn_b)
    res = run_bass_kernel_spmd(nc, in_maps, core_ids=list(range(NCORES)), **run_kwargs)
    shards = [res.results[c]["out"].reshape(BPC, S, H) for c in range(NCORES)]
    full = np.concatenate(shards, axis=0).astype(np.float32)
    kernel.last_results = res
    return full


# revision 16
# speedup vs baseline: 1.0610x; 1.0610x over previous
"""ACNN sparse-attention Trainium2 kernel (8 NeuronCores, pure data parallel).

Reference computation (per batch b, token s):
  windows[s, w]   = x[s + w - 2]           (zero-padded outside the sequence)
  q               = x[s] @ Wq
  kp[s']          = x[s'] @ Wk             (zero rows stay zero)
  scores[s, w]    = sum_h v[h] * tanh(kp[s + w - 2, h] + q[s, h])
  att             = softmax_w(scores)
  out[s]          = sum_w att[s, w] * (x[s + w - 2] @ cnn_W[w*F:(w+1)*F]) + b

Sharding: batch 16 -> 2 per core. Everything on-device is computed from
x^T ([F, tokens], bf16) so all matmul contractions sit on the partition dim;
the W=5 window shifts become free-dim column shifts of x^T / kp^T.
"""
import sys

sys.path.insert(0, "/opt/trn_rl_repo")

from contextlib import ExitStack

import numpy as np
import ml_dtypes

import concourse.bass as bass
import concourse.tile as tile
from concourse import mybir
from concourse.bass_utils import run_bass_kernel_spmd

def _install_ntff_hook_shim():
    """The image's antenv package lacks axon_hooks; recreate it so
    run_bass_kernel_spmd(trace=True) can capture NTFF profiles."""
    import types

    if "antenv.axon_hooks" in sys.modules:
        return
    mod = types.ModuleType("antenv.axon_hooks")
    mod._hook = None
    mod.set_axon_ntff_profile_hook = lambda h: setattr(mod, "_hook", h)
    mod.get_axon_ntff_profile_hook = lambda: mod._hook
    sys.modules["antenv.axon_hooks"] = mod
    try:
        sys.path.insert(0, "/root/.axon_site/trn_agent_boot")
        import trn_boot

        hook = trn_boot._ntff_profile_via_ctypes("/opt/axon/libaxon_pjrt.so")
        if hook is not None:
            mod._hook = hook
    except Exception:
        pass


_install_ntff_hook_shim()

F32 = mybir.dt.float32
BF16 = mybir.dt.bfloat16
AF = mybir.ActivationFunctionType
AX = mybir.AxisListType

B, S, F, H, W = 16, 2048, 512, 512, 5
PAD = W // 2
NCORES = 8
BPC = B // NCORES          # batches per core
T = BPC * S                # tokens per core (4096)
TB = 512                   # token block
NB = T // TB               # 8 blocks per core
NBSEG = S // TB            # 4 blocks per segment (batch)
FC = F // 128              # 4 f-chunks
HC = H // 128              # 4 h-chunks
SEGW = S + 2 * PAD         # padded segment width in SBUF (2052)
XW = BPC * SEGW            # padded token width in SBUF (4104)

_MAX_CTRL_WAITS = 1


def _patch_tile_drain():
    """walrus rejects >4 sem waits on one CTRL; spread the TileContext exit
    drain's waits over extra drain instructions."""
    if getattr(tile.TileContext, "_acnn_drain_patched", False):
        return
    from concourse.vector_clock import ScopedClock

    def _drain_and_barrier(self, tick_clock, wait_clock):
        nc = self.nc
        drain_inst = nc.sync.drain()
        wait_clock.add_sem_waits(
            drain_inst.ins, ScopedClock({None: tick_clock.global_clock})
        )
        si = drain_inst.ins.sync_info
        waits = list(si.on_wait) if si and si.on_wait else []
        if len(waits) > _MAX_CTRL_WAITS:
            si.on_wait = waits[:_MAX_CTRL_WAITS]
            rest = waits[_MAX_CTRL_WAITS:]
            for i in range(0, len(rest), _MAX_CTRL_WAITS):
                extra = nc.sync.drain()
                esi = extra.ins.sync_info
                chunk = rest[i : i + _MAX_CTRL_WAITS]
                if esi is None:
                    extra.ins.sync_info = type(si)(on_wait=chunk, on_update=[])
                else:
                    esi.on_wait = chunk
        nc.all_engine_barrier()
        popped = nc._tile_sem_poison_stack.pop()
        assert popped is self._sem_poison
        nc.clear_and_free_semaphores(list(self.sems.allocated().values()))
        nc.all_engine_barrier()

    tile.TileContext._drain_and_barrier = _drain_and_barrier
    tile.TileContext._acnn_drain_patched = True


def _split_multi_waits(nc, max_waits=_MAX_CTRL_WAITS):
    """This walrus build rejects >1 sem wait per compute instruction; hoist
    overflow waits onto same-engine NoOps placed just before (engine queues
    are strict FIFO, so the gating is equivalent)."""
    n_split = 0
    for f in nc.m.functions:
        for bb in f.blocks:
            new = []
            for inst in bb.instructions:
                si = inst.sync_info
                waits = list(si.on_wait) if si is not None and si.on_wait else []
                if len(waits) > max_waits:
                    extra = waits[: len(waits) - max_waits]
                    si.on_wait = waits[len(waits) - max_waits:]
                    for i in range(0, len(extra), max_waits):
                        nop = mybir.InstNoOp(
                            name=f"{inst.name}-xw{i}",
                            engine=inst.engine,
                            sync_info=mybir.SyncInfo(
                                on_wait=extra[i : i + max_waits], on_update=[]
                            ),
                            bass_nofuse=True,
                            ins=[],
                            outs=[],
                        )
                        new.append(nop)
                    n_split += 1
                new.append(inst)
            bb.instructions[:] = new
    return n_split


def build():
    _patch_tile_drain()
    nc = bass.Bass(trn_type="TRN2")

    AXW = 520  # per-f-chunk region width in the ax tiles (512 + shift fringe)

    xT = nc.dram_tensor("xT", [F, T], BF16, kind="ExternalInput")
    wq = nc.dram_tensor("wq", [F, H], BF16, kind="ExternalInput")
    wk = nc.dram_tensor("wk", [F, H], BF16, kind="ExternalInput")
    cw = nc.dram_tensor("cw", [W * F, H], BF16, kind="ExternalInput")
    vT = nc.dram_tensor("vT", [128, HC], BF16, kind="ExternalInput")
    bb = nc.dram_tensor("bb", [128, H], F32, kind="ExternalInput")
    ident = nc.dram_tensor("ident", [128, 128], BF16, kind="ExternalInput")
    out = nc.dram_tensor("out", [T, H], F32, kind="ExternalOutput")

    with ExitStack() as ctx:
        tc = ctx.enter_context(tile.TileContext(nc))

        const = ctx.enter_context(tc.tile_pool(name="const", bufs=1))
        wq_sb = const.tile([128, FC * H], BF16, tag="wq")
        wk_sb = const.tile([128, FC * H], BF16, tag="wk")
        cw_sb = const.tile([128, W * FC * H], BF16, tag="cw")
        vT_sb = const.tile([128, HC], BF16, tag="vT")
        bb_sb = const.tile([128, H], F32, tag="bb")
        id_sb = const.tile([128, 128], BF16, tag="ident")
        xt_sb = const.tile([128, FC * XW], BF16, tag="xt")
        kp_sb = [const.tile([128, XW], BF16, name=f"kp{hc}", tag=f"kp{hc}") for hc in range(HC)]
        q_sb = [const.tile([128, T], BF16, name=f"q{hc}", tag=f"q{hc}") for hc in range(HC)]

        def dma_xt(fc, seg, half=None):
            o0, o1 = (0, S) if half is None else (half * (S // 2), (half + 1) * (S // 2))
            nc.sync.dma_start(
                xt_sb[:, fc * XW + seg * SEGW + PAD + o0: fc * XW + seg * SEGW + PAD + o1],
                xT[fc * 128:(fc + 1) * 128, seg * S + o0: seg * S + o1],
            )

        # issue order matters: the first qkp matmuls need xt(seg0) + wq
        for fc in range(FC):
            dma_xt(fc, 0, 0)
            nc.sync.dma_start(wq_sb[:, fc * H:(fc + 1) * H], wq[fc * 128:(fc + 1) * 128, :])
        for fc in range(FC):
            dma_xt(fc, 0, 1)
        for fc in range(FC):
            nc.sync.dma_start(wk_sb[:, fc * H:(fc + 1) * H], wk[fc * 128:(fc + 1) * 128, :])
            dma_xt(fc, 1)
        nc.sync.dma_start(vT_sb[:], vT[:])
        nc.sync.dma_start(bb_sb[:], bb[:])
        nc.sync.dma_start(id_sb[:], ident[:])
        for w in range(W):
            for fc in range(FC):
                r0 = w * F + fc * 128
                c0 = (w * FC + fc) * H
                nc.sync.dma_start(cw_sb[:, c0:c0 + H], cw[r0:r0 + 128, :])
        # zero the halo columns of x^T and kp^T
        for seg in range(BPC):
            for fc in range(FC):
                b0 = fc * XW + seg * SEGW
                nc.vector.memset(xt_sb[:, b0:b0 + PAD], 0.0)
                nc.vector.memset(xt_sb[:, b0 + PAD + S:b0 + SEGW], 0.0)
            for hc in range(HC):
                nc.vector.memset(kp_sb[hc][:, seg * SEGW: seg * SEGW + PAD], 0.0)
                nc.vector.memset(kp_sb[hc][:, seg * SEGW + PAD + S: (seg + 1) * SEGW], 0.0)

        qkp_ps = ctx.enter_context(tc.tile_pool(name="qkp_ps", bufs=2, space="PSUM"))
        sc_ps_pool = ctx.enter_context(tc.tile_pool(name="sc_ps", bufs=1, space="PSUM"))
        tr_ps_pool = ctx.enter_context(tc.tile_pool(name="tr_ps", bufs=1, space="PSUM"))
        out_ps_pool = ctx.enter_context(tc.tile_pool(name="out_ps", bufs=1, space="PSUM"))

        argp = ctx.enter_context(tc.tile_pool(name="argp", bufs=2))
        thp = ctx.enter_context(tc.tile_pool(name="thp", bufs=5))
        smp = ctx.enter_context(tc.tile_pool(name="smp", bufs=4))
        attp = ctx.enter_context(tc.tile_pool(name="attp", bufs=2))
        bcp = ctx.enter_context(tc.tile_pool(name="bcp", bufs=4))
        axp = ctx.enter_context(tc.tile_pool(name="axp", bufs=3))
        outp = ctx.enter_context(tc.tile_pool(name="outp", bufs=4))

        def xcol0(b):
            return (b // NBSEG) * SEGW + PAD + (b % NBSEG) * TB

        th_tiles = {}
        attT_tiles = {}
        sc_tiles = {}

        def emit_qkp(b):
            xc = xcol0(b)
            qc = b * TB
            for hc in range(HC):
                for which, w_sb, dst, dc in (
                    (0, wq_sb, q_sb[hc], qc),
                    (1, wk_sb, kp_sb[hc], xc),
                ):
                    ps = qkp_ps.tile([128, TB], F32, name="ps", tag="qkp")
                    for fc in range(FC):
                        nc.tensor.matmul(
                            ps[:],
                            w_sb[:, fc * H + hc * 128: fc * H + (hc + 1) * 128],
                            xt_sb[:, fc * XW + xc: fc * XW + xc + TB],
                            start=(fc == 0),
                            stop=(fc == FC - 1),
                        )
                    if which == 0:
                        nc.scalar.activation(dst[:, dc:dc + TB], ps[:], AF.Copy)
                    else:
                        nc.vector.tensor_copy(dst[:, dc:dc + TB], ps[:])

        def emit_addstanh(b):
            xc = xcol0(b)
            qc = b * TB
            ths = []
            for hc in range(HC):
                arg = argp.tile([128, W * TB], BF16, name="arg", tag="arg")
                for w in range(W):
                    nc.vector.tensor_add(
                        arg[:, w * TB:(w + 1) * TB],
                        kp_sb[hc][:, xc - PAD + w: xc - PAD + w + TB],
                        q_sb[hc][:, qc:qc + TB],
                    )
                th = thp.tile([128, W * TB], BF16, name="th", tag="th")
                nc.scalar.activation(th[:], arg[:], AF.Tanh)
                ths.append(th)
            th_tiles[b] = ths

        def emit_scores(b):
            sc_ps = sc_ps_pool.tile([128, 32], F32, name="sc", tag="sc")
            ths = th_tiles.pop(b)
            for hc in range(HC):
                th = ths[hc]
                for w in range(W):
                    for g in range(4):
                        col = g * 8 + w
                        nc.tensor.matmul(
                            sc_ps[:, col:col + 1],
                            th[:, w * TB + g * 128: w * TB + (g + 1) * 128],
                            vT_sb[:, hc:hc + 1],
                            start=(hc == 0 and w == 0 and g == 0),
                            stop=(hc == HC - 1 and w == W - 1 and g == 3),
                            skip_group_check=True,
                        )
            # softmax over W=5 (scores bounded by |v|_1 ~ 8, no max-sub needed)
            attT = attp.tile([W, TB], BF16, name="attT", tag="attT")
            for g in range(4):
                ex = smp.tile([128, W], F32, name="ex", tag="ex")
                nc.scalar.activation(ex[:], sc_ps[:, g * 8: g * 8 + W], AF.Exp)
                sm = smp.tile([128, 1], F32, name="sm", tag="sm")
                nc.vector.reduce_sum(sm[:], ex[:], AX.X)
                rc = smp.tile([128, 1], F32, name="rc", tag="rc")
                nc.vector.reciprocal(rc[:], sm[:])
                attg = smp.tile([128, W], BF16, name="attg", tag="attg")
                nc.vector.tensor_scalar_mul(attg[:], ex[:], rc[:])
                tp = tr_ps_pool.tile([128, 128], BF16, name="tp", tag="tr")
                nc.tensor.transpose(tp[0:W, :], attg[:], id_sb[:])
                nc.scalar.activation(attT[:, g * 128:(g + 1) * 128], tp[0:W, :], AF.Copy)
            attT_tiles[b] = attT

        def emit_cnn(b):
            xc = xcol0(b)
            qc = b * TB
            attT = attT_tiles.pop(b)
            ops = [out_ps_pool.tile([128, H], F32, name=f"op{g}", tag=f"op{g}") for g in range(4)]
            abs_ = []
            for w in range(W):
                # att row w broadcast, written at column offset w so the
                # merged multiply below reads x^T at an even (2x-mode) offset
                ab = bcp.tile([128, AXW], BF16, name="ab", tag="ab")
                nc.sync.dma_start(
                    ab[:, w:w + TB],
                    attT[w:w + 1, :].rearrange("p (r c) -> p r c", r=1)
                    .broadcast_to((1, 128, TB)),
                )
                abs_.append(ab)
            for w in range(W):
                ab = abs_[w]
                ax = axp.tile([128, FC * AXW], BF16, name="ax", tag="ax")
                # ax[:, fc, j] = xt[:, fc, xc-2+j] * att_w[j-w]   (j in [0,516))
                nc.vector.tensor_tensor(
                    ax.rearrange("p (f c) -> p f c", f=FC)[:, :, 0:516],
                    xt_sb.rearrange("p (f c) -> p f c", f=FC)[:, :, xc - PAD: xc - PAD + 516],
                    ab[:, 0:516].rearrange("p (r c) -> p r c", r=1)
                    .broadcast_to((128, FC, 516)),
                    mybir.AluOpType.mult,
                )
                for fc in range(FC):
                    for g in range(4):
                        nc.tensor.matmul(
                            ops[g][:],
                            ax[:, fc * AXW + w + g * 128: fc * AXW + w + (g + 1) * 128],
                            cw_sb[:, (w * FC + fc) * H:(w * FC + fc + 1) * H],
                            start=(w == 0 and fc == 0),
                            stop=(w == W - 1 and fc == FC - 1),
                        )
            for g in range(4):
                ot = outp.tile([128, H], F32, name="ot", tag="ot")
                nc.vector.tensor_add(ot[:], ops[g][:], bb_sb[:])
                nc.sync.dma_start(out[qc + g * 128: qc + (g + 1) * 128, :], ot[:])

        # software-pipelined emission: per-engine program order is chosen so
        # ready PE work (next qkp / cnn of block b) is never queued behind
        # PE work that waits on the softmax chain of a newer block.
        emit_qkp(0)
        emit_qkp(1)
        emit_qkp(2)
        emit_addstanh(0)
        emit_scores(0)
        for b in range(NB):
            if b + 3 < NB:
                emit_qkp(b + 3)
            if b + 1 < NB:
                emit_addstanh(b + 1)
                emit_scores(b + 1)
            emit_cnn(b)

    _split_multi_waits(nc)
    return nc


_NC_CACHE = None


def _get_nc():
    global _NC_CACHE
    if _NC_CACHE is None:
        _NC_CACHE = build()
    return _NC_CACHE


def _prep_in_maps(embeds_output, Wq, Wk, v_att, cnn_W, cnn_b):
    bf = ml_dtypes.bfloat16
    wq = np.ascontiguousarray(Wq, dtype=np.float32).astype(bf)
    wk = np.ascontiguousarray(Wk, dtype=np.float32).astype(bf)
    cw = np.ascontiguousarray(cnn_W, dtype=np.float32).astype(bf)
    vT = np.ascontiguousarray(
        np.asarray(v_att, dtype=np.float32).reshape(HC, 128).T
    ).astype(bf)
    bb = np.ascontiguousarray(
        np.broadcast_to(np.asarray(cnn_b, dtype=np.float32)[None, :], (128, H))
    )
    ident = np.eye(128, dtype=np.float32).astype(bf)

    x = np.asarray(embeds_output, dtype=np.float32)
    in_maps = []
    for c in range(NCORES):
        shard = x[c * BPC:(c + 1) * BPC]                  # [BPC, S, F]
        xT = shard.transpose(2, 0, 1).reshape(F, T)       # [F, BPC*S]
        in_maps.append(
            {
                "xT": np.ascontiguousarray(xT).astype(bf),
                "wq": wq,
                "wk": wk,
                "cw": cw,
                "vT": vT,
                "bb": bb,
                "ident": ident,
            }
        )
    return in_maps


def kernel(embeds_output, Wq, Wk, v_att, cnn_W, cnn_b, **run_kwargs):
    nc = _get_nc()
    in_maps = _prep_in_maps(embeds_output, Wq, Wk, v_att, cnn_W, cnn_b)
    res = run_bass_kernel_spmd(nc, in_maps, core_ids=list(range(NCORES)), **run_kwargs)
    shards = [res.results[c]["out"].reshape(BPC, S, H) for c in range(NCORES)]
    full = np.concatenate(shards, axis=0).astype(np.float32)
    kernel.last_results = res
    return full


# revision 17
# speedup vs baseline: 1.2276x; 1.1570x over previous
"""ACNN sparse-attention Trainium2 kernel (8 NeuronCores, pure data parallel).

Reference computation (per batch b, token s):
  windows[s, w]   = x[s + w - 2]           (zero-padded outside the sequence)
  q               = x[s] @ Wq
  kp[s']          = x[s'] @ Wk             (zero rows stay zero)
  scores[s, w]    = sum_h v[h] * tanh(kp[s + w - 2, h] + q[s, h])
  att             = softmax_w(scores)
  out[s]          = sum_w att[s, w] * (x[s + w - 2] @ cnn_W[w*F:(w+1)*F]) + b

Sharding: batch 16 -> 2 per core. Everything on-device is computed from
x^T ([F, tokens], bf16) so all matmul contractions sit on the partition dim;
the W=5 window shifts become free-dim column shifts of x^T / kp^T.
"""
import sys

sys.path.insert(0, "/opt/trn_rl_repo")

from contextlib import ExitStack

import numpy as np
import ml_dtypes

import concourse.bass as bass
import concourse.tile as tile
from concourse import mybir
from concourse.bass_utils import run_bass_kernel_spmd

def _install_ntff_hook_shim():
    """The image's antenv package lacks axon_hooks; recreate it so
    run_bass_kernel_spmd(trace=True) can capture NTFF profiles."""
    import types

    if "antenv.axon_hooks" in sys.modules:
        return
    mod = types.ModuleType("antenv.axon_hooks")
    mod._hook = None
    mod.set_axon_ntff_profile_hook = lambda h: setattr(mod, "_hook", h)
    mod.get_axon_ntff_profile_hook = lambda: mod._hook
    sys.modules["antenv.axon_hooks"] = mod
    try:
        sys.path.insert(0, "/root/.axon_site/trn_agent_boot")
        import trn_boot

        hook = trn_boot._ntff_profile_via_ctypes("/opt/axon/libaxon_pjrt.so")
        if hook is not None:
            mod._hook = hook
    except Exception:
        pass


_install_ntff_hook_shim()

F32 = mybir.dt.float32
BF16 = mybir.dt.bfloat16
AF = mybir.ActivationFunctionType
AX = mybir.AxisListType

B, S, F, H, W = 16, 2048, 512, 512, 5
PAD = W // 2
NCORES = 8
BPC = B // NCORES          # batches per core
T = BPC * S                # tokens per core (4096)
TB = 512                   # token block
NB = T // TB               # 8 blocks per core
NBSEG = S // TB            # 4 blocks per segment (batch)
FC = F // 128              # 4 f-chunks
HC = H // 128              # 4 h-chunks
SEGW = S + 2 * PAD         # padded segment width in SBUF (2052)
XW = BPC * SEGW            # padded token width in SBUF (4104)

_MAX_CTRL_WAITS = 1


def _patch_tile_drain():
    """walrus rejects >4 sem waits on one CTRL; spread the TileContext exit
    drain's waits over extra drain instructions."""
    if getattr(tile.TileContext, "_acnn_drain_patched", False):
        return
    from concourse.vector_clock import ScopedClock

    def _drain_and_barrier(self, tick_clock, wait_clock):
        nc = self.nc
        drain_inst = nc.sync.drain()
        wait_clock.add_sem_waits(
            drain_inst.ins, ScopedClock({None: tick_clock.global_clock})
        )
        si = drain_inst.ins.sync_info
        waits = list(si.on_wait) if si and si.on_wait else []
        if len(waits) > _MAX_CTRL_WAITS:
            si.on_wait = waits[:_MAX_CTRL_WAITS]
            rest = waits[_MAX_CTRL_WAITS:]
            for i in range(0, len(rest), _MAX_CTRL_WAITS):
                extra = nc.sync.drain()
                esi = extra.ins.sync_info
                chunk = rest[i : i + _MAX_CTRL_WAITS]
                if esi is None:
                    extra.ins.sync_info = type(si)(on_wait=chunk, on_update=[])
                else:
                    esi.on_wait = chunk
        nc.all_engine_barrier()
        popped = nc._tile_sem_poison_stack.pop()
        assert popped is self._sem_poison
        nc.clear_and_free_semaphores(list(self.sems.allocated().values()))
        nc.all_engine_barrier()

    tile.TileContext._drain_and_barrier = _drain_and_barrier
    tile.TileContext._acnn_drain_patched = True


def _split_multi_waits(nc, max_waits=_MAX_CTRL_WAITS):
    """This walrus build rejects >1 sem wait per compute instruction; hoist
    overflow waits onto same-engine NoOps placed just before (engine queues
    are strict FIFO, so the gating is equivalent)."""
    n_split = 0
    for f in nc.m.functions:
        for bb in f.blocks:
            new = []
            for inst in bb.instructions:
                si = inst.sync_info
                waits = list(si.on_wait) if si is not None and si.on_wait else []
                if len(waits) > max_waits:
                    extra = waits[: len(waits) - max_waits]
                    si.on_wait = waits[len(waits) - max_waits:]
                    for i in range(0, len(extra), max_waits):
                        nop = mybir.InstNoOp(
                            name=f"{inst.name}-xw{i}",
                            engine=inst.engine,
                            sync_info=mybir.SyncInfo(
                                on_wait=extra[i : i + max_waits], on_update=[]
                            ),
                            bass_nofuse=True,
                            ins=[],
                            outs=[],
                        )
                        new.append(nop)
                    n_split += 1
                new.append(inst)
            bb.instructions[:] = new
    return n_split


def build():
    _patch_tile_drain()
    nc = bass.Bass(trn_type="TRN2")

    AXW = 520  # per-f-chunk region width in the ax tiles (512 + shift fringe)

    xT = nc.dram_tensor("xT", [F, T], BF16, kind="ExternalInput")
    wq = nc.dram_tensor("wq", [F, H], BF16, kind="ExternalInput")
    wk = nc.dram_tensor("wk", [F, H], BF16, kind="ExternalInput")
    cw = nc.dram_tensor("cw", [W * F, H], BF16, kind="ExternalInput")
    vT = nc.dram_tensor("vT", [128, HC], BF16, kind="ExternalInput")
    bb = nc.dram_tensor("bb", [128, H], F32, kind="ExternalInput")
    ident = nc.dram_tensor("ident", [128, 128], BF16, kind="ExternalInput")
    out = nc.dram_tensor("out", [T, H], F32, kind="ExternalOutput")

    with ExitStack() as ctx:
        tc = ctx.enter_context(tile.TileContext(nc))

        const = ctx.enter_context(tc.tile_pool(name="const", bufs=1))
        wq_sb = const.tile([128, FC * H], BF16, tag="wq")
        wk_sb = const.tile([128, FC * H], BF16, tag="wk")
        cw_sb = const.tile([128, W * FC * H], BF16, tag="cw")
        vT_sb = const.tile([128, HC], BF16, tag="vT")
        bb_sb = const.tile([128, H], F32, tag="bb")
        id_sb = const.tile([128, 128], BF16, tag="ident")
        xt_sb = const.tile([128, FC * XW], BF16, tag="xt")
        kp_sb = [const.tile([128, XW], BF16, name=f"kp{hc}", tag=f"kp{hc}") for hc in range(HC)]
        q_sb = [const.tile([128, T], BF16, name=f"q{hc}", tag=f"q{hc}") for hc in range(HC)]

        def dma_xt(fc, seg, half=None):
            o0, o1 = (0, S) if half is None else (half * (S // 2), (half + 1) * (S // 2))
            nc.sync.dma_start(
                xt_sb[:, fc * XW + seg * SEGW + PAD + o0: fc * XW + seg * SEGW + PAD + o1],
                xT[fc * 128:(fc + 1) * 128, seg * S + o0: seg * S + o1],
            )

        # issue order matters: the first qkp matmuls need xt(seg0) + wq
        for fc in range(FC):
            dma_xt(fc, 0)
            nc.sync.dma_start(wq_sb[:, fc * H:(fc + 1) * H], wq[fc * 128:(fc + 1) * 128, :])
        for fc in range(FC):
            nc.sync.dma_start(wk_sb[:, fc * H:(fc + 1) * H], wk[fc * 128:(fc + 1) * 128, :])
            dma_xt(fc, 1)
        nc.sync.dma_start(vT_sb[:], vT[:])
        nc.sync.dma_start(bb_sb[:], bb[:])
        nc.sync.dma_start(id_sb[:], ident[:])
        for w in range(W):
            for fc in range(FC):
                r0 = w * F + fc * 128
                c0 = (w * FC + fc) * H
                nc.sync.dma_start(cw_sb[:, c0:c0 + H], cw[r0:r0 + 128, :])
        # zero the halo columns of x^T and kp^T
        for seg in range(BPC):
            for fc in range(FC):
                b0 = fc * XW + seg * SEGW
                nc.vector.memset(xt_sb[:, b0:b0 + PAD], 0.0)
                nc.vector.memset(xt_sb[:, b0 + PAD + S:b0 + SEGW], 0.0)
            for hc in range(HC):
                nc.vector.memset(kp_sb[hc][:, seg * SEGW: seg * SEGW + PAD], 0.0)
                nc.vector.memset(kp_sb[hc][:, seg * SEGW + PAD + S: (seg + 1) * SEGW], 0.0)

        qkp_ps = ctx.enter_context(tc.tile_pool(name="qkp_ps", bufs=2, space="PSUM"))
        sc_ps_pool = ctx.enter_context(tc.tile_pool(name="sc_ps", bufs=1, space="PSUM"))
        tr_ps_pool = ctx.enter_context(tc.tile_pool(name="tr_ps", bufs=1, space="PSUM"))
        out_ps_pool = ctx.enter_context(tc.tile_pool(name="out_ps", bufs=1, space="PSUM"))

        argp = ctx.enter_context(tc.tile_pool(name="argp", bufs=2))
        thp = ctx.enter_context(tc.tile_pool(name="thp", bufs=5))
        smp = ctx.enter_context(tc.tile_pool(name="smp", bufs=4))
        attp = ctx.enter_context(tc.tile_pool(name="attp", bufs=2))
        bcp = ctx.enter_context(tc.tile_pool(name="bcp", bufs=4))
        axp = ctx.enter_context(tc.tile_pool(name="axp", bufs=3))
        outp = ctx.enter_context(tc.tile_pool(name="outp", bufs=4))

        def xcol0(b):
            return (b // NBSEG) * SEGW + PAD + (b % NBSEG) * TB

        th_tiles = {}
        attT_tiles = {}
        sc_tiles = {}

        def emit_qkp(b):
            xc = xcol0(b)
            qc = b * TB
            for hc in range(HC):
                for which, w_sb, dst, dc in (
                    (0, wq_sb, q_sb[hc], qc),
                    (1, wk_sb, kp_sb[hc], xc),
                ):
                    ps = qkp_ps.tile([128, TB], F32, name="ps", tag="qkp")
                    for fc in range(FC):
                        nc.tensor.matmul(
                            ps[:],
                            w_sb[:, fc * H + hc * 128: fc * H + (hc + 1) * 128],
                            xt_sb[:, fc * XW + xc: fc * XW + xc + TB],
                            start=(fc == 0),
                            stop=(fc == FC - 1),
                        )
                    if which == 0:
                        nc.scalar.activation(dst[:, dc:dc + TB], ps[:], AF.Copy)
                    else:
                        nc.vector.tensor_copy(dst[:, dc:dc + TB], ps[:])

        def emit_addstanh(b):
            xc = xcol0(b)
            qc = b * TB
            ths = []
            for hc in range(HC):
                arg = argp.tile([128, W * TB], BF16, name="arg", tag="arg")
                for w in range(W):
                    nc.vector.tensor_add(
                        arg[:, w * TB:(w + 1) * TB],
                        kp_sb[hc][:, xc - PAD + w: xc - PAD + w + TB],
                        q_sb[hc][:, qc:qc + TB],
                    )
                th = thp.tile([128, W * TB], BF16, name="th", tag="th")
                nc.scalar.activation(th[:], arg[:], AF.Tanh)
                ths.append(th)
            th_tiles[b] = ths

        def emit_scores(b):
            sc_ps = sc_ps_pool.tile([128, 32], F32, name="sc", tag="sc")
            ths = th_tiles.pop(b)
            for hc in range(HC):
                th = ths[hc]
                for w in range(W):
                    for g in range(4):
                        col = g * 8 + w
                        nc.tensor.matmul(
                            sc_ps[:, col:col + 1],
                            th[:, w * TB + g * 128: w * TB + (g + 1) * 128],
                            vT_sb[:, hc:hc + 1],
                            start=(hc == 0 and w == 0 and g == 0),
                            stop=(hc == HC - 1 and w == W - 1 and g == 3),
                            skip_group_check=True,
                        )
            # softmax over W=5 (scores bounded by |v|_1 ~ 8, no max-sub needed)
            attT = attp.tile([W, TB], BF16, name="attT", tag="attT")
            for g in range(4):
                ex = smp.tile([128, W], F32, name="ex", tag="ex")
                nc.scalar.activation(ex[:], sc_ps[:, g * 8: g * 8 + W], AF.Exp)
                sm = smp.tile([128, 1], F32, name="sm", tag="sm")
                nc.vector.reduce_sum(sm[:], ex[:], AX.X)
                rc = smp.tile([128, 1], F32, name="rc", tag="rc")
                nc.vector.reciprocal(rc[:], sm[:])
                attg = smp.tile([128, W], BF16, name="attg", tag="attg")
                nc.vector.tensor_scalar_mul(attg[:], ex[:], rc[:])
                tp = tr_ps_pool.tile([128, 128], BF16, name="tp", tag="tr")
                nc.tensor.transpose(tp[0:W, :], attg[:], id_sb[:])
                nc.scalar.activation(attT[:, g * 128:(g + 1) * 128], tp[0:W, :], AF.Copy)
            attT_tiles[b] = attT

        def emit_cnn(b):
            xc = xcol0(b)
            qc = b * TB
            attT = attT_tiles.pop(b)
            ops = [out_ps_pool.tile([128, H], F32, name=f"op{g}", tag=f"op{g}") for g in range(4)]
            abs_ = []
            for w in range(W):
                # att row w broadcast, written at column offset w so the
                # merged multiply below reads x^T at an even (2x-mode) offset
                ab = bcp.tile([128, AXW], BF16, name="ab", tag="ab")
                nc.sync.dma_start(
                    ab[:, w:w + TB],
                    attT[w:w + 1, :].rearrange("p (r c) -> p r c", r=1)
                    .broadcast_to((1, 128, TB)),
                )
                abs_.append(ab)
            for w in range(W):
                ab = abs_[w]
                ax = axp.tile([128, FC * AXW], BF16, name="ax", tag="ax")
                # ax[:, fc, j] = xt[:, fc, xc-2+j] * att_w[j-w]   (j in [0,516))
                nc.vector.tensor_tensor(
                    ax.rearrange("p (f c) -> p f c", f=FC)[:, :, 0:516],
                    xt_sb.rearrange("p (f c) -> p f c", f=FC)[:, :, xc - PAD: xc - PAD + 516],
                    ab[:, 0:516].rearrange("p (r c) -> p r c", r=1)
                    .broadcast_to((128, FC, 516)),
                    mybir.AluOpType.mult,
                )
                for fc in range(FC):
                    for g in range(4):
                        nc.tensor.matmul(
                            ops[g][:],
                            ax[:, fc * AXW + w + g * 128: fc * AXW + w + (g + 1) * 128],
                            cw_sb[:, (w * FC + fc) * H:(w * FC + fc + 1) * H],
                            start=(w == 0 and fc == 0),
                            stop=(w == W - 1 and fc == FC - 1),
                        )
            for g in range(4):
                ot = outp.tile([128, H], F32, name="ot", tag="ot")
                nc.vector.tensor_add(ot[:], ops[g][:], bb_sb[:])
                nc.sync.dma_start(out[qc + g * 128: qc + (g + 1) * 128, :], ot[:])

        # software-pipelined emission: per-engine program order is chosen so
        # ready PE work (next qkp / cnn of block b) is never queued behind
        # PE work that waits on the softmax chain of a newer block.
        emit_qkp(0)
        emit_qkp(1)
        emit_qkp(2)
        emit_addstanh(0)
        emit_scores(0)
        for b in range(NB):
            if b + 3 < NB:
                emit_qkp(b + 3)
            if b + 1 < NB:
                emit_addstanh(b + 1)
                emit_scores(b + 1)
            emit_cnn(b)

    _split_multi_waits(nc)
    return nc


_NC_CACHE = None


def _get_nc():
    global _NC_CACHE
    if _NC_CACHE is None:
        _NC_CACHE = build()
    return _NC_CACHE


def _prep_in_maps(embeds_output, Wq, Wk, v_att, cnn_W, cnn_b):
    bf = ml_dtypes.bfloat16
    wq = np.ascontiguousarray(Wq, dtype=np.float32).astype(bf)
    wk = np.ascontiguousarray(Wk, dtype=np.float32).astype(bf)
    cw = np.ascontiguousarray(cnn_W, dtype=np.float32).astype(bf)
    vT = np.ascontiguousarray(
        np.asarray(v_att, dtype=np.float32).reshape(HC, 128).T
    ).astype(bf)
    bb = np.ascontiguousarray(
        np.broadcast_to(np.asarray(cnn_b, dtype=np.float32)[None, :], (128, H))
    )
    ident = np.eye(128, dtype=np.float32).astype(bf)

    x = np.asarray(embeds_output, dtype=np.float32)
    in_maps = []
    for c in range(NCORES):
        shard = x[c * BPC:(c + 1) * BPC]                  # [BPC, S, F]
        xT = shard.transpose(2, 0, 1).reshape(F, T)       # [F, BPC*S]
        in_maps.append(
            {
                "xT": np.ascontiguousarray(xT).astype(bf),
                "wq": wq,
                "wk": wk,
                "cw": cw,
                "vT": vT,
                "bb": bb,
                "ident": ident,
            }
        )
    return in_maps


def kernel(embeds_output, Wq, Wk, v_att, cnn_W, cnn_b, **run_kwargs):
    nc = _get_nc()
    in_maps = _prep_in_maps(embeds_output, Wq, Wk, v_att, cnn_W, cnn_b)
    res = run_bass_kernel_spmd(nc, in_maps, core_ids=list(range(NCORES)), **run_kwargs)
    shards = [res.results[c]["out"].reshape(BPC, S, H) for c in range(NCORES)]
    full = np.concatenate(shards, axis=0).astype(np.float32)
    kernel.last_results = res
    return full
